# revision 31
# baseline (speedup 1.0000x reference)
"""Trainium2 Bass kernel: dot-product attention scores (matvec).

scores = encoder_out[16384, 4096] @ decoder_hidden[-1][4096] -> [16384]

Sharding: encoder_out row-wise across 8 cores (2048 rows each),
decoder_hidden replicated. No cross-core communication.

Per-core kernel (memory-bound, 32 MB of HBM reads), raw Bass with
manual semaphores (the TileContext tail drain does not compile with
this walrus build, and the fused raw-ISA DVE ops — tensor_tensor_reduce,
affine_mul_reduce, partition_broadcast — are rejected by its codegen,
so the compute is split across standard-BIR ops on two engines):

  sync (SP/HWDGE):  one 2 MB broadcast of t[4096] to all 128
                    partitions (step-0 DRAM source AP), then streams
                    encoder rows as 16 x [128, 4096] tiles (2 MB per
                    dma_start, 8 slots deep), then the final store.
                    The whole stream is fabric-bound at ~430 GB/s.
  vector (DVE):     one in-place tensor_mul per block: ebuf *= tb.
                    In-place removes separate product buffers; ACT
                    reads the ebuf directly.
  scalar (ACT):     per block activation(Copy, accum_out) -> the
                    per-partition sum over 4096 free elems = the dot
                    products.

Sem protocol notes (learned from races on HW under profiling):
  - one DMA sem per enc buffer slot; a single cumulatively-counted sem
    is racy because each dma_start's 16 incs come from 16 independent
    SDMA engines, so sem >= 16*(i+1) can be hit while transfer i still
    has a lagging engine
  - cross-engine handoffs (ACT accum -> store DMA) must go through a
    semaphore; issuing the store from the producing engine right after
    the producing op raced on HW
  - slot reuse order: ACT consumes the product (red_sem) -> sync
    reissues the slot's DMA -> DVE's esem wait covers the in-place
    overwrite

Per-core output is [128, 16] with out[p, n] = scores[n*128 + p];
the host transposes/flattens and concatenates the 8 shards.
"""

import numpy as np

S, H, L = 16384, 4096, 2
N_CORES = 8
S_LOC = S // N_CORES        # 2048 rows per core
P = 128                     # SBUF partitions
N_BLOCKS = S_LOC // P       # 16 row-blocks per core = 16 loads of 2 MB
NBUF = 8                    # enc tile slots

_NC_CACHE = {}
LAST_RESULT = None          # BassKernelResults of the most recent run


def _build_nc():
    import concourse.bass as bass
    from concourse import mybir

    f32 = mybir.dt.float32

    nc = bass.Bass(trn_type="TRN2")
    enc = nc.dram_tensor("enc", [S_LOC, H], f32, kind="ExternalInput")
    dec = nc.dram_tensor("dec", [L, H], f32, kind="ExternalInput")
    out = nc.dram_tensor("out", [P, N_BLOCKS], f32, kind="ExternalOutput")

    # enc row r = n*P + p  ->  [n, p, h]; per-partition one contiguous 16 KB run
    enc_r = enc.rearrange("(n p) h -> n p h", p=P)

    from contextlib import ExitStack

    with ExitStack() as ctx:
        tb = ctx.enter_context(nc.sbuf_tensor("tb", [P, H], f32))
        ebufs = [
            ctx.enter_context(nc.sbuf_tensor(f"ebuf{i}", [P, H], f32))
            for i in range(NBUF)
        ]
        junk = ctx.enter_context(nc.sbuf_tensor("junk", [P, H], mybir.dt.bfloat16))
        sc = ctx.enter_context(nc.sbuf_tensor("sc", [P, N_BLOCKS], f32))
        tb_sem = ctx.enter_context(nc.semaphore("tb_sem"))
        esems = [ctx.enter_context(nc.semaphore(f"esem{i}")) for i in range(NBUF)]
        mul_sem = ctx.enter_context(nc.semaphore("mul_sem"))
        red_sem = ctx.enter_context(nc.semaphore("red_sem"))
        store_sem = ctx.enter_context(nc.semaphore("store_sem"))
        block = ctx.enter_context(nc.Block())

        @block.sync
        def _(sync):
            sync.dma_start(tb[:], dec[L - 1 : L, :].to_broadcast((P, H))).then_inc(
                tb_sem, 16
            )
            for i in range(N_BLOCKS):
                if i >= NBUF:
                    # slot reuse: ACT must have consumed block i-NBUF (the
                    # in-place product lives in the same slot)
                    sync.wait_ge(red_sem, i - NBUF + 1)
                sync.dma_start(ebufs[i % NBUF][:], enc_r[i]).then_inc(
                    esems[i % NBUF], 16
                )
            sync.wait_ge(red_sem, N_BLOCKS)
            sync.dma_start(out[:], sc[:]).then_inc(store_sem, 16)
            sync.wait_ge(store_sem, 16)

        @block.vector
        def _(vector):
            vector.wait_ge(tb_sem, 16)
            for n in range(N_BLOCKS):
                vector.wait_ge(esems[n % NBUF], 16 * (n // NBUF + 1))
                eb = ebufs[n % NBUF][:]
                nc.vector.tensor_mul(eb, eb, tb[:]).then_inc(mul_sem, 1)

        @block.scalar
        def _(scalar):
            # warm the ACT function table while idle (lazy-loads ~1.3 us on
            # first ACTIVATE otherwise)
            nc.scalar.activation(
                out=junk[0:1, 0:1],
                in_=junk[0:1, 0:1],
                func=mybir.ActivationFunctionType.Copy,
            )
            for n in range(N_BLOCKS):
                scalar.wait_ge(mul_sem, n + 1)
                nc.scalar.activation(
                    out=junk[:],
                    in_=ebufs[n % NBUF][:],
                    func=mybir.ActivationFunctionType.Copy,
                    accum_out=sc[:, n : n + 1],
                ).then_inc(red_sem, 1)

    return nc


def kernel(encoder_out: np.ndarray, decoder_hidden: np.ndarray) -> np.ndarray:
    global LAST_RESULT
    from concourse.bass_utils import run_bass_kernel_spmd

    encoder_out = np.ascontiguousarray(np.asarray(encoder_out, dtype=np.float32))
    decoder_hidden = np.ascontiguousarray(np.asarray(decoder_hidden, dtype=np.float32))

    if "nc" not in _NC_CACHE:
        _NC_CACHE["nc"] = _build_nc()
    nc = _NC_CACHE["nc"]

    in_maps = [
        {"enc": encoder_out[c * S_LOC : (c + 1) * S_LOC], "dec": decoder_hidden}
        for c in range(N_CORES)
    ]
    res = run_bass_kernel_spmd(nc, in_maps, core_ids=list(range(N_CORES)))
    LAST_RESULT = res

    # out[p, n] = scores[n*128 + p] within each shard
    parts = [np.asarray(r["out"]).T.reshape(-1) for r in res.results]
    return np.concatenate(parts).astype(np.float32)


# revision 33
# speedup vs baseline: 1.1165x; 1.1165x over previous
"""Trainium2 Bass kernel: dot-product attention scores (matvec).

scores = encoder_out[16384, 4096] @ decoder_hidden[-1][4096] -> [16384]

Sharding: encoder_out row-wise across 8 cores (2048 rows each),
decoder_hidden replicated. No cross-core communication.

Per-core kernel (memory-bound, 32 MB of HBM reads), raw Bass with
manual semaphores (the TileContext tail drain does not compile with
this walrus build, and the fused raw-ISA DVE ops — tensor_tensor_reduce,
affine_mul_reduce, partition_broadcast — are rejected by its codegen,
so the compute is split across standard-BIR ops on two engines):

  sync (SP/HWDGE):  one 2 MB broadcast of t[4096] to all 128
                    partitions (step-0 DRAM source AP), then streams
                    encoder rows as [2,2,2,2,2,2,2,1,1] row-blocks per
                    dma_start (4 MB pairs amortize the per-transfer
                    completion-receipt latency that inflates under HBM
                    contention; 2 MB singles keep the tail short),
                    then the final store. The stream is fabric-bound
                    at ~430 GB/s.
  vector (DVE):     duplicates t on-chip to [P, 2H] = t||t during the
                    idle pipeline-fill window, then one in-place 2-D
                    tensor_mul per load: ebuf *= t||t. In-place
                    removes separate product buffers; ACT reads the
                    ebuf directly.
  scalar (ACT):     per 128-row block activation(Copy, accum_out) ->
                    the per-partition sum over 4096 free elems = the
                    dot products.

Sem protocol notes (learned from races on HW under profiling):
  - one DMA sem per enc buffer slot; a single cumulatively-counted sem
    is racy because each dma_start's 16 incs come from 16 independent
    SDMA engines, so sem >= 16*(i+1) can be hit while transfer i still
    has a lagging engine
  - cross-engine handoffs (ACT accum -> store DMA) must go through a
    semaphore; issuing the store from the producing engine right after
    the producing op raced on HW
  - slot reuse order: ACT consumes the product (red_sem) -> sync
    reissues the slot's DMA -> DVE's esem wait covers the in-place
    overwrite

Per-core output is [128, 16] with out[p, n] = scores[n*128 + p];
the host transposes/flattens and concatenates the 8 shards.
"""

import numpy as np

S, H, L = 16384, 4096, 2
N_CORES = 8
S_LOC = S // N_CORES        # 2048 rows per core
P = 128                     # SBUF partitions
N_BLOCKS = S_LOC // P       # 16 row-blocks per core
LOADS = [2, 2, 2, 2, 2, 2, 2, 1, 1]   # blocks per dma_start; sum == 16
MAXC = max(LOADS)
NBUF = 4                    # enc tile slots, each [P, MAXC*H]

_NC_CACHE = {}
LAST_RESULT = None          # BassKernelResults of the most recent run

assert sum(LOADS) == N_BLOCKS
CUM = [0]
for c in LOADS:
    CUM.append(CUM[-1] + c)


def _build_nc():
    import concourse.bass as bass
    from concourse import mybir

    f32 = mybir.dt.float32

    nc = bass.Bass(trn_type="TRN2")
    enc = nc.dram_tensor("enc", [S_LOC, H], f32, kind="ExternalInput")
    dec = nc.dram_tensor("dec", [L, H], f32, kind="ExternalInput")
    out = nc.dram_tensor("out", [P, N_BLOCKS], f32, kind="ExternalOutput")

    # enc row r = n*P + p  ->  [p, n, h]; a load of c blocks reads, per
    # partition, c contiguous 16 KB runs 2 MB apart
    enc_r = enc.rearrange("(n p) h -> p n h", p=P)

    from contextlib import ExitStack

    with ExitStack() as ctx:
        # tb2 holds t twice so a 2-block tile multiplies against one
        # contiguous 2-D operand
        tb2 = ctx.enter_context(nc.sbuf_tensor("tb2", [P, MAXC * H], f32))
        ebufs = [
            ctx.enter_context(nc.sbuf_tensor(f"ebuf{i}", [P, MAXC * H], f32))
            for i in range(NBUF)
        ]
        junk = ctx.enter_context(nc.sbuf_tensor("junk", [P, H], mybir.dt.bfloat16))
        sc = ctx.enter_context(nc.sbuf_tensor("sc", [P, N_BLOCKS], f32))
        tb_sem = ctx.enter_context(nc.semaphore("tb_sem"))
        tb2_sem = ctx.enter_context(nc.semaphore("tb2_sem"))
        esems = [ctx.enter_context(nc.semaphore(f"esem{i}")) for i in range(NBUF)]
        mul_sem = ctx.enter_context(nc.semaphore("mul_sem"))
        red_sem = ctx.enter_context(nc.semaphore("red_sem"))
        store_sem = ctx.enter_context(nc.semaphore("store_sem"))
        block = ctx.enter_context(nc.Block())

        @block.sync
        def _(sync):
            sync.dma_start(
                tb2[:, :H], dec[L - 1 : L, :].to_broadcast((P, H))
            ).then_inc(tb_sem, 16)
            for l, c in enumerate(LOADS):
                if l >= NBUF:
                    # slot reuse: ACT must have consumed every block of
                    # load l-NBUF (the in-place product lives there)
                    sync.wait_ge(red_sem, CUM[l - NBUF + 1])
                src = enc_r[:, CUM[l] : CUM[l] + c, :]
                dst = ebufs[l % NBUF][:, : c * H].rearrange("p (c h) -> p c h", c=c)
                sync.dma_start(dst, src).then_inc(esems[l % NBUF], 16)
            sync.wait_ge(red_sem, N_BLOCKS)
            sync.dma_start(out[:], sc[:]).then_inc(store_sem, 16)
            sync.wait_ge(store_sem, 16)

        @block.vector
        def _(vector):
            vector.wait_ge(tb_sem, 16)
            # duplicate t into the upper half during the pipeline-fill
            # idle window (f32 copy runs in DVE 2x mode); same-engine
            # ordering covers the later reads
            nc.vector.tensor_copy(tb2[:, H:], tb2[:, :H])
            for l, c in enumerate(LOADS):
                vector.wait_ge(esems[l % NBUF], 16 * (l // NBUF + 1))
                eb = ebufs[l % NBUF][:, : c * H]
                nc.vector.tensor_mul(eb, eb, tb2[:, : c * H]).then_inc(mul_sem, 1)

        @block.scalar
        def _(scalar):
            # warm the ACT function table while idle (lazy-loads ~1.3 us on
            # first ACTIVATE otherwise)
            nc.scalar.activation(
                out=junk[0:1, 0:1],
                in_=junk[0:1, 0:1],
                func=mybir.ActivationFunctionType.Copy,
            )
            for l, c in enumerate(LOADS):
                scalar.wait_ge(mul_sem, l + 1)
                for j in range(c):
                    n = CUM[l] + j
                    nc.scalar.activation(
                        out=junk[:],
                        in_=ebufs[l % NBUF][:, j * H : (j + 1) * H],
                        func=mybir.ActivationFunctionType.Copy,
                        accum_out=sc[:, n : n + 1],
                    ).then_inc(red_sem, 1)

    return nc


def kernel(encoder_out: np.ndarray, decoder_hidden: np.ndarray) -> np.ndarray:
    global LAST_RESULT
    from concourse.bass_utils import run_bass_kernel_spmd

    encoder_out = np.ascontiguousarray(np.asarray(encoder_out, dtype=np.float32))
    decoder_hidden = np.ascontiguousarray(np.asarray(decoder_hidden, dtype=np.float32))

    if "nc" not in _NC_CACHE:
        _NC_CACHE["nc"] = _build_nc()
    nc = _NC_CACHE["nc"]

    in_maps = [
        {"enc": encoder_out[c * S_LOC : (c + 1) * S_LOC], "dec": decoder_hidden}
        for c in range(N_CORES)
    ]
    res = run_bass_kernel_spmd(nc, in_maps, core_ids=list(range(N_CORES)))
    LAST_RESULT = res

    # out[p, n] = scores[n*128 + p] within each shard
    parts = [np.asarray(r["out"]).T.reshape(-1) for r in res.results]
    return np.concatenate(parts).astype(np.float32)
